# revision 16
# baseline (speedup 1.0000x reference)
"""CoTrackerThreeOnline corr-embedding kernel for 8x Trainium2 NeuronCores.

Sharding: data-parallel over the N=1024 tracks (128 per core).
Host (numpy): shards inputs, gathers + bilinear-samples the fmap pyramid at
the 7x7 support offsets (pure data staging / layout transform), and computes
the tiny rel-posenc tail. Device (Bass/Tile): per-track correlation volumes
(49x49 per frame/level), the 2401->384 gelu MLP, 384->256 projection, bias +
time-embedding add -- i.e. all the matmul-heavy compute (~11 GFLOP/core).

Device schedule: a single software pipeline over 16 "units" (4 levels x 4
groups of 32 tracks). Unit u's corr matmuls are emitted 2 units ahead of the
MLP1 that consumes them, so the tensor queue interleaves
  corr(u) | MLP1(half (u-2)/2) | MLP2(half (u-3)/2)
and DMA prefetch (16-track sampt subtiles, 2 units ahead) keeps the PE fed
continuously instead of stalling at level boundaries.

Device layout notes:
- Corr volume for track n, level l: out[ij, (hw, s)] = track_n^T @ samp_n.
  The 49 hw values are split 0-24 / 25-48 and the two halves are written to
  PSUM partitions 0-48 and 64-112 (matmul tile_position=(0,64)), so the
  49x49=2401 contraction dim of the MLP presents as 25 chunks of 128
  partitions (98 real rows each) with w1 zero-padded on the unused rows.
- PSUM corr tiles come from a pool (fresh tile per pair); drains alternate
  vector (even pair) / scalar (odd pair) engines.
- Output DMAs are issued from the gpsimd queue so they never head-of-line
  block the input loads on the sync queue.
- Token (column) order per 512-token half: tok = uu*256 + pl*16 + t*8 + s
  where the track is n = 32*(2*h + uu) + 2*pl + t. The host unscrambles.
"""

import numpy as np
import ml_dtypes

BF16 = ml_dtypes.bfloat16

R = 3
STRIDE = 4
RES = (384, 512)
G = 2 * R + 1          # 7
GG = G * G             # 49
B, S, N, C = 1, 8, 1024, 128
H0, W0 = RES[0] // STRIDE, RES[1] // STRIDE   # 96, 128
NCORES = 8
NPC = N // NCORES      # 128 tracks per core
NT = 16                # tracks per staged sample subtile
NP25 = 25              # hw-pair chunks (hw p and hw 25+p share a 128-row K chunk)


def _bilinear_sample(fmap, x, y):
    """Exact numpy port of reference.bilinear_sample. fmap: (BT,C,H,W)."""
    BT, Cc, H, W = fmap.shape
    x0f = np.floor(x)
    y0f = np.floor(y)
    wx = (x - x0f)[:, None, :].astype(np.float32)
    wy = (y - y0f)[:, None, :].astype(np.float32)
    x0 = np.clip(x0f.astype(np.int32), 0, W - 1)
    x1 = np.clip(x0f.astype(np.int32) + 1, 0, W - 1)
    y0 = np.clip(y0f.astype(np.int32), 0, H - 1)
    y1 = np.clip(y0f.astype(np.int32) + 1, 0, H - 1)
    flat = fmap.reshape(BT, Cc, H * W)

    def g(yi, xi):
        idx = (yi * W + xi)[:, None, :]
        return np.take_along_axis(flat, idx, axis=2)

    return (g(y0, x0) * (1 - wx) * (1 - wy) + g(y0, x1) * wx * (1 - wy)
            + g(y1, x0) * (1 - wx) * wy + g(y1, x1) * wx * wy)


def _posenc(x):
    scales = np.asarray([2.0 ** i for i in range(10)], np.float32)
    xb = (x[..., None, :] * scales[:, None]).reshape(x.shape[:-1] + (-1,))
    four = np.sin(np.concatenate([xb, xb + 0.5 * np.pi], axis=-1))
    return np.concatenate([x, four], axis=-1)


def _stage_sampled(fmaps, coords):
    """Bilinear-sample all levels -> sampT (4, N, C, S, 49) float32.

    sampT[l, n, c, t, hw] = corr_feat of reference (hw = i*7+j grid index).
    """
    d = np.linspace(-R, R, G).astype(np.float32)
    xoff, yoff = np.meshgrid(d, d, indexing="ij")   # (7,7) rows=x off
    xoff = xoff.reshape(-1)
    yoff = yoff.reshape(-1)
    out = np.empty((4, N, C, S, GG), np.float32)
    for lvl in range(4):
        fm = fmaps[lvl]                 # (1, S, C, H, W)
        _, _, _, H, W = fm.shape
        c = coords.reshape(S, N, 1, 2) / (2.0 ** lvl)
        x = (c[..., 0] + xoff[None, None, :]).reshape(S, N * GG)
        y = (c[..., 1] + yoff[None, None, :]).reshape(S, N * GG)
        samp = _bilinear_sample(fm.reshape(S, C, H, W), x, y)  # (S, C, N*GG)
        samp = samp.reshape(S, C, N, GG)
        out[lvl] = samp.transpose(2, 1, 0, 3)       # (N, C, S, GG)
    return out


def _build_device_program():
    import concourse.bacc as bacc
    import concourse.tile as tile
    from concourse import mybir

    f32 = mybir.dt.float32
    bf16 = mybir.dt.bfloat16

    nc = bacc.Bacc(None)
    # DRAM params (per-core shapes)
    # sampt cols: hw*8+s for hw 0..48, padded to 400 (cols 392:400 zero)
    sampt = nc.declare_dram_parameter("sampt", [4, 8, C, NT, 400], bf16, isOutput=False)
    trackt = nc.declare_dram_parameter("trackt", [4, C, NPC, GG], bf16, isOutput=False)
    # w1p[ij, p, m] = w1[p*49+ij, m]; w1p[64+ij, p, m] = w1[(25+p)*49+ij, m]
    # (p<24); all other rows zero.
    w1p = nc.declare_dram_parameter("w1p", [C, NP25, 384], bf16, isOutput=False)
    w2s = nc.declare_dram_parameter("w2s", [C, 3, 256], bf16, isOutput=False)
    b1s = nc.declare_dram_parameter("b1s", [C, 3], f32, isOutput=False)
    # te3[p, lvl, m2, s] = time_emb[s, 2 + lvl*256 + m2*128 + p] + b2[...]
    te3 = nc.declare_dram_parameter("te3", [C, 4, 2, S], f32, isOutput=False)
    # OUT[feat, tok]: feat = lvl*256 + m2*128 + p,
    # tok = h*512 + uu*256 + pl*16 + t*8 + s ; track n = 32*(2h+uu) + 2pl + t
    OUT = nc.declare_dram_parameter("OUT", [1024, NPC * S], f32, isOutput=True)

    NUNITS = 16            # 4 levels x 4 groups of 32 tracks
    NPAIR = 16             # track pairs per unit

    with tile.TileContext(nc) as tc:
        with (
            tc.tile_pool(name="const", bufs=1) as cpool,
            tc.tile_pool(name="track", bufs=3) as tpool,
            tc.tile_pool(name="sampt", bufs=8) as spool,
            tc.tile_pool(name="c2", bufs=2) as c2pool,
            tc.tile_pool(name="hsb", bufs=2) as hpool,
            tc.tile_pool(name="osb", bufs=2) as opool,
            tc.tile_pool(name="pg", bufs=4, space="PSUM") as pg,
            tc.tile_pool(name="ph", bufs=2, space="PSUM") as ph,
            tc.tile_pool(name="pe", bufs=2, space="PSUM") as pe,
        ):
            # Zero the pg pool's physical slots once (warmup tiles alias the
            # per-pair tiles below): partitions 49-63/113-127 are never
            # written by the corr matmuls, and every drain copy propagates
            # their zeros into corr2 padding.
            for _ in range(4):
                g2w = pg.tile([C, 2, 32, 8], f32, name="g2")
                nc.vector.memset(g2w[:], 0.0)

            st_tiles = [None] * (NUNITS * 2)     # 16-track sampt subtiles
            trk_tiles = [None] * 8               # 64-track trackt half-tiles
            c2_tiles = [None] * 8                # per-half corr2 tiles
            hs_tiles = [None] * 8

            def load_st(ti, split=False):
                lp, tl = ti // 8, ti % 8
                st = spool.tile([C, NT, 400], bf16, name="st")
                st_tiles[ti] = st
                if split:
                    nc.sync.dma_start(st[:, 0:8], sampt[lp, tl, :, 0:8])
                    nc.sync.dma_start(st[:, 8:16], sampt[lp, tl, :, 8:16])
                else:
                    nc.sync.dma_start(st[:], sampt[lp, tl])

            def load_trk(hk):
                lp, n0 = hk // 2, (hk % 2) * 64
                trk = tpool.tile([C, 64, GG], bf16, name="trk")
                trk_tiles[hk] = trk
                nc.sync.dma_start(trk[:], trackt[lp, :, n0:n0 + 64])

            # ---- startup DMAs, ordered for just-in-time ramp arrival.
            # Each dma_start costs ~0.65us of serial issue time on the sync
            # queue, so keep this list short; rings stream concurrently. ----
            trk0 = tpool.tile([C, 64, GG], bf16, name="trk")
            trk_tiles[0] = trk0
            nc.sync.dma_start(trk0[:, 0:16], trackt[0, :, 0:16])
            st0 = spool.tile([C, NT, 400], bf16, name="st")
            st_tiles[0] = st0
            nc.sync.dma_start(st0[:, 0:4], sampt[0, 0, :, 0:4])
            nc.sync.dma_start(st0[:, 4:16], sampt[0, 0, :, 4:16])
            nc.sync.dma_start(trk0[:, 16:64], trackt[0, :, 16:64])
            load_st(1)
            w1_sb = cpool.tile([C, NP25, 384], bf16)
            nc.sync.dma_start(w1_sb[:, :, 0:128], w1p[:, :, 0:128])
            load_st(2)
            nc.sync.dma_start(w1_sb[:, :, 128:384], w1p[:, :, 128:384])
            load_st(3)
            b1_sb = cpool.tile([C, 3], f32)
            nc.sync.dma_start(b1_sb[:], b1s[:])
            w2_sb = cpool.tile([C, 3, 256], bf16)
            nc.sync.dma_start(w2_sb[:], w2s[:])
            te_sb = cpool.tile([C, 4, 2, S], f32)
            nc.sync.dma_start(te_sb[:], te3[:])

            def corr_pair(i, pl):
                """One track pair of unit i (2 matmuls + drain)."""
                l, g = i // 4, i % 4
                k, uu = i // 2, i % 2
                c2 = c2_tiles[k]
                trk = trk_tiles[i // 2]
                g2 = pg.tile([C, 2, 32, 8], f32, name="g2")
                for t in range(2):
                    nloc = 32 * (g % 2) + 2 * pl + t
                    st = st_tiles[2 * i + (2 * pl + t) // NT]
                    nq = (2 * pl + t) % NT
                    # hw 0..24 -> partitions 0..48
                    nc.tensor.matmul(
                        g2[0:49, t, 0:25, :],
                        trk[:, nloc],
                        st[:, nq, 0:200],
                        start=True, stop=True,
                    )
                    # hw 25..48 -> partitions 64..112
                    nc.tensor.matmul(
                        g2[64:113, t, 0:24, :],
                        trk[:, nloc],
                        st[:, nq, 200:392],
                        start=True, stop=True,
                    )
                # drain the pair split across both engines: halves the
                # per-pair drain latency so deep corr bursts don't stall
                # on PSUM slot reuse
                src = g2[:, :, 0:25, :].transpose([0, 2, 1, 3])
                nc.vector.tensor_copy(c2[:, 0:13, uu, pl], src[:, 0:13])
                nc.scalar.activation(
                    c2[:, 13:25, uu, pl], src[:, 13:25],
                    mybir.ActivationFunctionType.Copy)

            def mlp1_mms(k, uu=None):
                """Yield the MLP1 matmul (+gelu) closures for half k in
                chain order; uu=None -> 512-token chains, else the
                256-token unit slice (ramp)."""
                c2 = c2_tiles[k]
                if uu is None or uu == 0:
                    hs_tiles[k] = hpool.tile([C, 3, 512], bf16, name="hs")
                hs = hs_tiles[k]
                hh = [None]

                def mk_mm(m, p):
                    def go():
                        if p == 0:
                            hh[0] = ph.tile([C, 512], f32, name="hh")
                        if uu is None:
                            dst, src, ncol = hs[:, m], c2[:, p], 512
                        else:
                            dst = hs[:, m, uu * 256:(uu + 1) * 256]
                            src, ncol = c2[:, p, uu], 256
                        nc.tensor.matmul(
                            hh[0][:, 0:ncol],
                            w1_sb[:, p, m * 128:(m + 1) * 128],
                            src,
                            start=(p == 0), stop=(p == NP25 - 1),
                        )
                        if p == NP25 - 1:
                            nc.scalar.activation(
                                dst, hh[0][:, 0:ncol],
                                mybir.ActivationFunctionType.Gelu,
                                bias=b1_sb[:, m:m + 1],
                            )
                    return go
                return [mk_mm(m, p) for m in range(3) for p in range(NP25)]

            def mlp2(k):
                l, h = k // 2, k % 2
                hs = hs_tiles[k]
                for m2 in range(2):
                    ee = pe.tile([128, 512], f32)
                    for kk in range(3):
                        nc.tensor.matmul(
                            ee[:],
                            w2_sb[:, kk, m2 * 128:(m2 + 1) * 128],
                            hs[:, kk],
                            start=(kk == 0), stop=(kk == 2),
                        )
                    osb = opool.tile([128, 512], f32)
                    nc.vector.tensor_tensor(
                        osb[:], ee[:],
                        te_sb[:, l, m2, :].unsqueeze(1)
                        .broadcast_to((C, 64, S)),
                        mybir.AluOpType.add,
                    )
                    f0 = l * 256 + m2 * 128
                    nc.gpsimd.dma_start(
                        OUT[f0: f0 + 128, h * 512:(h + 1) * 512],
                        osb[:],
                    )

            # ---- interleaved stretch pipeline over 8 halves ----
            # Stretch 0: corr halves H0 (units 0,1), DMA-paced.  Stretch
            # k>=1: MLP2 H(k-2), then the 32 corr pairs of H_k interleaved
            # 2:5 with the 75 MLP1 matmuls of H(k-1) -- the MLP1 stream
            # gives the drain engines room so corr never stalls on PSUM.
            # Tail: MLP2 H6, MLP1 H7, MLP2 H7.
            def prefetch(k):
                for u_pre in (2 * k + 2, 2 * k + 3):
                    if u_pre < NUNITS:
                        if trk_tiles[u_pre // 2] is None:
                            load_trk(u_pre // 2)
                        for j in (0, 1):
                            if st_tiles[2 * u_pre + j] is None:
                                load_st(2 * u_pre + j)

            def interleave(pairs, mms, delay=0):
                """Emit `pairs` [(unit, pl), ...] in granules of 8, with
                the mm closures spread between granules (starting after
                `delay` granules). Coarse granules amortize the ~0.3us
                PE pipeline bubble at each corr<->MLP transition while
                still pacing the drains."""
                nsteps = len(pairs) // 8
                idx = 0
                for s in range(nsteps):
                    for u, pl in pairs[8 * s:8 * s + 8]:
                        corr_pair(u, pl)
                    if s >= delay:
                        take = (len(mms) - idx) // (nsteps - s)
                        for f in mms[idx:idx + take]:
                            f()
                        idx += take
                for f in mms[idx:]:
                    f()

            def unit_pairs(*units):
                return [(u, pl) for u in units for pl in range(NPAIR)]

            # Stretch 0 (ramp, DMA-bound): u0 plain, then u1 interleaved
            # with the 256-col MLP1 of u0 so the PE has work while sampt
            # streams in.
            c2_tiles[0] = c2pool.tile([C, NP25, 2, NPAIR, 2, S], bf16,
                                      name="c2")
            prefetch(0)
            for pl in range(NPAIR):
                corr_pair(0, pl)
            interleave(unit_pairs(1), mlp1_mms(0, uu=0), delay=1)

            # Stretch 1: corr H1 interleaved with MLP1 of u1.
            prefetch(1)
            c2_tiles[1] = c2pool.tile([C, NP25, 2, NPAIR, 2, S], bf16,
                                      name="c2")
            interleave(unit_pairs(2, 3), mlp1_mms(0, uu=1), delay=1)

            for k in range(2, 8):
                prefetch(k)
                mlp2(k - 2)
                c2_tiles[k] = c2pool.tile([C, NP25, 2, NPAIR, 2, S], bf16,
                                          name="c2")
                interleave(unit_pairs(2 * k, 2 * k + 1), mlp1_mms(k - 1))
            mlp2(6)
            for f in mlp1_mms(7):
                f()
            mlp2(7)
    nc.finalize()
    return nc


_NC_CACHE = {}


def kernel(**inputs):
    fmaps = [np.asarray(inputs[f"fmaps{i}"], np.float32) for i in range(4)]
    tracks = [np.asarray(inputs[f"track{i}"], np.float32) for i in range(4)]
    coords = np.asarray(inputs["coords"], np.float32)
    vis = np.asarray(inputs["vis"], np.float32)
    conf = np.asarray(inputs["conf"], np.float32)
    w1 = np.asarray(inputs["w1"], np.float32)
    b1 = np.asarray(inputs["b1"], np.float32)
    w2 = np.asarray(inputs["w2"], np.float32)
    b2 = np.asarray(inputs["b2"], np.float32)
    time_emb = np.asarray(inputs["time_emb"], np.float32)

    # ---- host staging ----
    sampT = _stage_sampled(fmaps, coords)          # (4, N, C, S, 49) f32

    # w1 viewed as (49 hw, 49 ij, 384) -> packed K chunks of 128
    w1v = w1.reshape(GG, GG, 384)
    w1p_full = np.zeros((C, NP25, 384), np.float32)
    w1p_full[0:49] = w1v[0:25].transpose(1, 0, 2)
    w1p_full[64:113, 0:24] = w1v[25:49].transpose(1, 0, 2)
    w1p_full = w1p_full.astype(BF16)
    w2s_full = np.ascontiguousarray(
        w2.reshape(3, 128, 256).transpose(1, 0, 2)).astype(BF16)
    b1s_full = np.ascontiguousarray(b1.reshape(3, 128).T).astype(np.float32)
    te_slice = time_emb[0, :, 2:1026] + np.tile(b2, 4)[None, :]   # (S, 1024)
    # te3[p, lvl, m2, s]
    te3_full = np.ascontiguousarray(
        te_slice.T.reshape(4, 2, 128, S).transpose(2, 0, 1, 3)).astype(np.float32)

    in_maps = []
    for k in range(NCORES):
        ns = slice(k * NPC, (k + 1) * NPC)
        # sampt: (4 lvl, 8 tile, C, 16 n, 400) cols hw*8+s (zero pad 392:400)
        sa = sampT[:, ns]                              # (4, NPC, C, S, GG)
        sa = sa.transpose(0, 2, 1, 4, 3)               # (lvl, c, n, hw, s)
        sa = sa.reshape(4, C, 8, NT, GG * S)
        sa = np.concatenate(
            [sa, np.zeros((4, C, 8, NT, 8), np.float32)], axis=-1)
        sa = np.ascontiguousarray(sa.transpose(0, 2, 1, 3, 4)).astype(BF16)
        # trackt: (4, C, NPC, 49); track lvl input (1, 49, N, C)
        tr = np.stack([
            np.ascontiguousarray(t[0][:, ns].transpose(2, 1, 0))
            for t in tracks
        ]).astype(BF16)
        in_maps.append({
            "sampt": sa,
            "trackt": tr,
            "w1p": w1p_full,
            "w2s": w2s_full,
            "b1s": b1s_full,
            "te3": te3_full,
        })

    # ---- device run ----
    from concourse import bass_utils
    global _LAST_INMAPS
    _LAST_INMAPS = in_maps
    if "nc" not in _NC_CACHE:
        _NC_CACHE["nc"] = _build_device_program()
    res = bass_utils.run_bass_kernel_spmd(
        _NC_CACHE["nc"], in_maps, list(range(NCORES)))
    results = res.results

    # ---- host tail: rel posenc + assembly ----
    rel_f = np.concatenate(
        [coords[:, :-1] - coords[:, 1:], np.zeros((1, 1, N, 2), np.float32)], axis=1)
    rel_b = np.concatenate(
        [np.zeros((1, 1, N, 2), np.float32), coords[:, 1:] - coords[:, :-1]], axis=1)
    scale = np.asarray([RES[1], RES[0]], np.float32) / STRIDE
    rel_emb = _posenc(np.concatenate(
        [rel_f / scale, rel_b / scale], axis=-1))     # (1, S, N, 84)

    out = np.empty((1, N, S, 1110), np.float32)
    te = time_emb[0]                                  # (S, 1110)
    out[0, :, :, 0] = vis[0, :, :, 0].T + te[None, :, 0]
    out[0, :, :, 1] = conf[0, :, :, 0].T + te[None, :, 1]
    out[0, :, :, 1026:] = rel_emb[0].transpose(1, 0, 2) + te[None, :, 1026:]
    for k in range(NCORES):
        # OUT[feat, tok]: feat = (lvl, m2, p), tok = (h, uu, pl, t, s);
        # track n = 32*(2h+uu) + 2pl + t
        o = np.asarray(results[k]["OUT"], np.float32)
        emb = o.reshape(1024, 2, 2, 16, 2, 8)          # (feat, h, uu, pl, t, s)
        emb = emb.transpose(1, 2, 3, 4, 5, 0).reshape(NPC, S, 1024)
        out[0, k * NPC:(k + 1) * NPC, :, 2:1026] = emb
    return out


# revision 21
# speedup vs baseline: 1.0725x; 1.0725x over previous
"""CoTrackerThreeOnline corr-embedding kernel for 8x Trainium2 NeuronCores.

Sharding: data-parallel over the N=1024 tracks (128 per core).
Host (numpy): shards inputs, gathers + bilinear-samples the fmap pyramid at
the 7x7 support offsets (pure data staging / layout transform), and computes
the tiny rel-posenc tail. Device (Bass/Tile): per-track correlation volumes
(49x49 per frame/level), the 2401->384 gelu MLP, 384->256 projection, bias +
time-embedding add -- i.e. all the matmul-heavy compute (~11 GFLOP/core).

Device schedule: a single software pipeline over 8 "halves" (64 tracks =
512 tokens each). Each steady-state stretch emits MLP2 of half k-2, then
the 32 corr pairs of half k interleaved (granules of 4 pairs : ~9 MLP
matmuls) with the MLP1 of half k-1. The interleave matters twice over:
corr pairs burst at ~85ns/track using both PE column groups (h0/h64
concurrently), which outruns the PSUM->SBUF drain engines 3x, so pairs
emitted back-to-back stall on PSUM slot reuse -- spreading them through
the MLP1 stream keeps both the PE and the drains saturated. DMA prefetch
runs 2 units ahead; the DMA-bound ramp (halves 0-1) instead interleaves
per-unit 256-col MLP1 chains with mms-first granule order so a
DMA-stalled pair never head-of-line blocks ready MLP work.

Device layout notes:
- Corr volume for track n, level l: out[ij, (hw, s)] = track_n^T @ samp_n.
  The 49 hw values are split 0-24 / 25-48 and the two halves are written to
  PSUM partitions 0-48 and 64-112 (matmul tile_position=(0,64)), so the
  49x49=2401 contraction dim of the MLP presents as 25 chunks of 128
  partitions (98 real rows each) with w1 zero-padded on the unused rows.
- PSUM corr tiles come from a pool (fresh tile per pair); drains alternate
  vector (even pair) / scalar (odd pair) engines.
- Output DMAs are issued from the gpsimd queue so they never head-of-line
  block the input loads on the sync queue.
- Token (column) order per 512-token half: tok = uu*256 + pl*16 + t*8 + s
  where the track is n = 32*(2*h + uu) + 2*pl + t. The host unscrambles.
"""

import numpy as np
import ml_dtypes

BF16 = ml_dtypes.bfloat16

R = 3
STRIDE = 4
RES = (384, 512)
G = 2 * R + 1          # 7
GG = G * G             # 49
B, S, N, C = 1, 8, 1024, 128
H0, W0 = RES[0] // STRIDE, RES[1] // STRIDE   # 96, 128
NCORES = 8
NPC = N // NCORES      # 128 tracks per core
NT = 16                # tracks per staged sample subtile
NP25 = 25              # hw-pair chunks (hw p and hw 25+p share a 128-row K chunk)


def _bilinear_sample(fmap, x, y):
    """Exact numpy port of reference.bilinear_sample. fmap: (BT,C,H,W)."""
    BT, Cc, H, W = fmap.shape
    x0f = np.floor(x)
    y0f = np.floor(y)
    wx = (x - x0f)[:, None, :].astype(np.float32)
    wy = (y - y0f)[:, None, :].astype(np.float32)
    x0 = np.clip(x0f.astype(np.int32), 0, W - 1)
    x1 = np.clip(x0f.astype(np.int32) + 1, 0, W - 1)
    y0 = np.clip(y0f.astype(np.int32), 0, H - 1)
    y1 = np.clip(y0f.astype(np.int32) + 1, 0, H - 1)
    flat = fmap.reshape(BT, Cc, H * W)

    def g(yi, xi):
        idx = (yi * W + xi)[:, None, :]
        return np.take_along_axis(flat, idx, axis=2)

    return (g(y0, x0) * (1 - wx) * (1 - wy) + g(y0, x1) * wx * (1 - wy)
            + g(y1, x0) * (1 - wx) * wy + g(y1, x1) * wx * wy)


def _posenc(x):
    scales = np.asarray([2.0 ** i for i in range(10)], np.float32)
    xb = (x[..., None, :] * scales[:, None]).reshape(x.shape[:-1] + (-1,))
    four = np.sin(np.concatenate([xb, xb + 0.5 * np.pi], axis=-1))
    return np.concatenate([x, four], axis=-1)


def _stage_sampled(fmaps, coords):
    """Bilinear-sample all levels -> sampT (4, N, C, S, 49) float32.

    sampT[l, n, c, t, hw] = corr_feat of reference (hw = i*7+j grid index).
    """
    d = np.linspace(-R, R, G).astype(np.float32)
    xoff, yoff = np.meshgrid(d, d, indexing="ij")   # (7,7) rows=x off
    xoff = xoff.reshape(-1)
    yoff = yoff.reshape(-1)
    out = np.empty((4, N, C, S, GG), np.float32)
    for lvl in range(4):
        fm = fmaps[lvl]                 # (1, S, C, H, W)
        _, _, _, H, W = fm.shape
        c = coords.reshape(S, N, 1, 2) / (2.0 ** lvl)
        x = (c[..., 0] + xoff[None, None, :]).reshape(S, N * GG)
        y = (c[..., 1] + yoff[None, None, :]).reshape(S, N * GG)
        samp = _bilinear_sample(fm.reshape(S, C, H, W), x, y)  # (S, C, N*GG)
        samp = samp.reshape(S, C, N, GG)
        out[lvl] = samp.transpose(2, 1, 0, 3)       # (N, C, S, GG)
    return out


def _build_device_program():
    import concourse.bacc as bacc
    import concourse.tile as tile
    from concourse import mybir

    f32 = mybir.dt.float32
    bf16 = mybir.dt.bfloat16

    nc = bacc.Bacc(None)
    # DRAM params (per-core shapes)
    # sampt cols: hw*8+s for hw 0..48, padded to 400 (cols 392:400 zero)
    sampt = nc.declare_dram_parameter("sampt", [4, 8, C, NT, 400], bf16, isOutput=False)
    trackt = nc.declare_dram_parameter("trackt", [4, C, NPC, GG], bf16, isOutput=False)
    # w1p[ij, p, m] = w1[p*49+ij, m]; w1p[64+ij, p, m] = w1[(25+p)*49+ij, m]
    # (p<24); all other rows zero.
    w1p = nc.declare_dram_parameter("w1p", [C, NP25, 384], bf16, isOutput=False)
    w2s = nc.declare_dram_parameter("w2s", [C, 3, 256], bf16, isOutput=False)
    b1s = nc.declare_dram_parameter("b1s", [C, 3], f32, isOutput=False)
    # te3[p, lvl, m2, s] = time_emb[s, 2 + lvl*256 + m2*128 + p] + b2[...]
    te3 = nc.declare_dram_parameter("te3", [C, 4, 2, S], f32, isOutput=False)
    # OUT[feat, tok]: feat = lvl*256 + m2*128 + p,
    # tok = h*512 + uu*256 + pl*16 + t*8 + s ; track n = 32*(2h+uu) + 2pl + t
    OUT = nc.declare_dram_parameter("OUT", [1024, NPC * S], f32, isOutput=True)

    NUNITS = 16            # 4 levels x 4 groups of 32 tracks
    NPAIR = 16             # track pairs per unit

    with tile.TileContext(nc) as tc:
        with (
            tc.tile_pool(name="const", bufs=1) as cpool,
            tc.tile_pool(name="track", bufs=3) as tpool,
            tc.tile_pool(name="sampt", bufs=8) as spool,
            tc.tile_pool(name="c2", bufs=2) as c2pool,
            tc.tile_pool(name="hsb", bufs=2) as hpool,
            tc.tile_pool(name="osb", bufs=2) as opool,
            tc.tile_pool(name="pg", bufs=4, space="PSUM") as pg,
            tc.tile_pool(name="ph", bufs=2, space="PSUM") as ph,
            tc.tile_pool(name="pe", bufs=2, space="PSUM") as pe,
        ):
            # Zero the pg pool's physical slots once (warmup tiles alias the
            # per-pair tiles below): partitions 49-63/113-127 are never
            # written by the corr matmuls, and every drain copy propagates
            # their zeros into corr2 padding.
            for _ in range(4):
                g2w = pg.tile([C, 2, 32, 8], f32, name="g2")
                nc.vector.memset(g2w[:], 0.0)

            st_tiles = [None] * (NUNITS * 2)     # 16-track sampt subtiles
            trk_tiles = [None] * 8               # 64-track trackt half-tiles
            c2_tiles = [None] * 8                # per-half corr2 tiles
            hs_tiles = [None] * 8

            def load_st(ti, split=False):
                lp, tl = ti // 8, ti % 8
                st = spool.tile([C, NT, 400], bf16, name="st")
                st_tiles[ti] = st
                if split:
                    nc.sync.dma_start(st[:, 0:8], sampt[lp, tl, :, 0:8])
                    nc.sync.dma_start(st[:, 8:16], sampt[lp, tl, :, 8:16])
                else:
                    nc.sync.dma_start(st[:], sampt[lp, tl])

            def load_trk(hk):
                lp, n0 = hk // 2, (hk % 2) * 64
                trk = tpool.tile([C, 64, GG], bf16, name="trk")
                trk_tiles[hk] = trk
                nc.sync.dma_start(trk[:], trackt[lp, :, n0:n0 + 64])

            # ---- startup DMAs, ordered for just-in-time ramp arrival.
            # Each dma_start costs ~0.65us of serial issue time on the sync
            # queue, so keep this list short; rings stream concurrently. ----
            trk0 = tpool.tile([C, 64, GG], bf16, name="trk")
            trk_tiles[0] = trk0
            nc.sync.dma_start(trk0[:, 0:16], trackt[0, :, 0:16])
            st0 = spool.tile([C, NT, 400], bf16, name="st")
            st_tiles[0] = st0
            nc.sync.dma_start(st0[:, 0:4], sampt[0, 0, :, 0:4])
            nc.sync.dma_start(st0[:, 4:16], sampt[0, 0, :, 4:16])
            nc.sync.dma_start(trk0[:, 16:64], trackt[0, :, 16:64])
            load_st(1)
            w1_sb = cpool.tile([C, NP25, 384], bf16)
            nc.sync.dma_start(w1_sb[:, :, 0:128], w1p[:, :, 0:128])
            nc.sync.dma_start(w1_sb[:, :, 128:384], w1p[:, :, 128:384])
            load_st(2)
            load_st(3)
            b1_sb = cpool.tile([C, 3], f32)
            nc.sync.dma_start(b1_sb[:], b1s[:])
            w2_sb = cpool.tile([C, 3, 256], bf16)
            nc.sync.dma_start(w2_sb[:], w2s[:])
            te_sb = cpool.tile([C, 4, 2, S], f32)
            nc.sync.dma_start(te_sb[:], te3[:])

            def corr_pair(i, pl):
                """One track pair of unit i (2 matmuls + drain)."""
                l, g = i // 4, i % 4
                k, uu = i // 2, i % 2
                c2 = c2_tiles[k]
                trk = trk_tiles[i // 2]
                g2 = pg.tile([C, 2, 32, 8], f32, name="g2")
                for t in range(2):
                    nloc = 32 * (g % 2) + 2 * pl + t
                    st = st_tiles[2 * i + (2 * pl + t) // NT]
                    nq = (2 * pl + t) % NT
                    # hw 0..24 -> partitions 0..48
                    nc.tensor.matmul(
                        g2[0:49, t, 0:25, :],
                        trk[:, nloc],
                        st[:, nq, 0:200],
                        start=True, stop=True,
                    )
                    # hw 25..48 -> partitions 64..112
                    nc.tensor.matmul(
                        g2[64:113, t, 0:24, :],
                        trk[:, nloc],
                        st[:, nq, 200:392],
                        start=True, stop=True,
                    )
                # drain whole pair; alternate engines
                src = g2[:, :, 0:25, :].transpose([0, 2, 1, 3])
                if pl % 2 == 0:
                    nc.vector.tensor_copy(c2[:, :, uu, pl], src)
                else:
                    nc.scalar.activation(
                        c2[:, :, uu, pl], src,
                        mybir.ActivationFunctionType.Copy)

            def mlp1_mms(k, uu=None):
                """Yield the MLP1 matmul (+gelu) closures for half k in
                chain order; uu=None -> 512-token chains, else the
                256-token unit slice (ramp)."""
                c2 = c2_tiles[k]
                if uu is None or uu == 0:
                    hs_tiles[k] = hpool.tile([C, 3, 512], bf16, name="hs")
                hs = hs_tiles[k]
                hh = [None]

                def mk_mm(m, p):
                    def go():
                        if p == 0:
                            hh[0] = ph.tile([C, 512], f32, name="hh")
                        if uu is None:
                            dst, src, ncol = hs[:, m], c2[:, p], 512
                        else:
                            dst = hs[:, m, uu * 256:(uu + 1) * 256]
                            src, ncol = c2[:, p, uu], 256
                        nc.tensor.matmul(
                            hh[0][:, 0:ncol],
                            w1_sb[:, p, m * 128:(m + 1) * 128],
                            src,
                            start=(p == 0), stop=(p == NP25 - 1),
                        )
                        if p == NP25 - 1:
                            nc.scalar.activation(
                                dst, hh[0][:, 0:ncol],
                                mybir.ActivationFunctionType.Gelu,
                                bias=b1_sb[:, m:m + 1],
                            )
                    return go
                return [mk_mm(m, p) for m in range(3) for p in range(NP25)]

            def mlp2(k):
                l, h = k // 2, k % 2
                hs = hs_tiles[k]
                for m2 in range(2):
                    ee = pe.tile([128, 512], f32)
                    for kk in range(3):
                        nc.tensor.matmul(
                            ee[:],
                            w2_sb[:, kk, m2 * 128:(m2 + 1) * 128],
                            hs[:, kk],
                            start=(kk == 0), stop=(kk == 2),
                        )
                    osb = opool.tile([128, 512], f32)
                    nc.vector.tensor_tensor(
                        osb[:], ee[:],
                        te_sb[:, l, m2, :].unsqueeze(1)
                        .broadcast_to((C, 64, S)),
                        mybir.AluOpType.add,
                    )
                    f0 = l * 256 + m2 * 128
                    nc.gpsimd.dma_start(
                        OUT[f0: f0 + 128, h * 512:(h + 1) * 512],
                        osb[:],
                    )

            # ---- interleaved stretch pipeline over 8 halves ----
            # Stretch 0: corr halves H0 (units 0,1), DMA-paced.  Stretch
            # k>=1: MLP2 H(k-2), then the 32 corr pairs of H_k interleaved
            # 2:5 with the 75 MLP1 matmuls of H(k-1) -- the MLP1 stream
            # gives the drain engines room so corr never stalls on PSUM.
            # Tail: MLP2 H6, MLP1 H7, MLP2 H7.
            def prefetch(k):
                for u_pre in (2 * k + 2, 2 * k + 3):
                    if u_pre < NUNITS:
                        if trk_tiles[u_pre // 2] is None:
                            load_trk(u_pre // 2)
                        for j in (0, 1):
                            if st_tiles[2 * u_pre + j] is None:
                                load_st(2 * u_pre + j)

            def interleave(pairs, mms, delay=0, mm_first=False):
                """Emit `pairs` [(unit, pl), ...] in granules of 4, with
                the mm closures spread between granules (starting after
                `delay` granules). Coarse granules amortize the ~0.3us
                PE pipeline bubble at each corr<->MLP transition while
                still pacing the drains. mm_first puts each granule's mms
                ahead of its pairs (for the DMA-bound ramp, where a pair
                stalled on sampt arrival must not block ready MLP work
                behind it in the in-order queue)."""
                nsteps = len(pairs) // 4
                idx = 0
                for s in range(nsteps):
                    take = 0
                    if s >= delay:
                        take = (len(mms) - idx) // (nsteps - s)
                    if mm_first:
                        for f in mms[idx:idx + take]:
                            f()
                    for u, pl in pairs[4 * s:4 * s + 4]:
                        corr_pair(u, pl)
                    if not mm_first:
                        for f in mms[idx:idx + take]:
                            f()
                    idx += take
                for f in mms[idx:]:
                    f()

            def unit_pairs(*units):
                return [(u, pl) for u in units for pl in range(NPAIR)]

            # Stretch 0 (ramp, DMA-bound): u0 plain, then u1 interleaved
            # with the 256-col MLP1 of u0 so the PE has work while sampt
            # streams in.
            c2_tiles[0] = c2pool.tile([C, NP25, 2, NPAIR, 2, S], bf16,
                                      name="c2")
            prefetch(0)
            for pl in range(NPAIR):
                corr_pair(0, pl)
            interleave(unit_pairs(1), mlp1_mms(0, uu=0), delay=1, mm_first=True)

            # Stretch 1: corr H1 interleaved with MLP1 of u1.
            prefetch(1)
            c2_tiles[1] = c2pool.tile([C, NP25, 2, NPAIR, 2, S], bf16,
                                      name="c2")
            interleave(unit_pairs(2, 3), mlp1_mms(0, uu=1), delay=1, mm_first=True)

            for k in range(2, 8):
                prefetch(k)
                mlp2(k - 2)
                c2_tiles[k] = c2pool.tile([C, NP25, 2, NPAIR, 2, S], bf16,
                                          name="c2")
                interleave(unit_pairs(2 * k, 2 * k + 1), mlp1_mms(k - 1))
            mlp2(6)
            for f in mlp1_mms(7):
                f()
            mlp2(7)
    nc.finalize()
    return nc


_NC_CACHE = {}


def kernel(**inputs):
    fmaps = [np.asarray(inputs[f"fmaps{i}"], np.float32) for i in range(4)]
    tracks = [np.asarray(inputs[f"track{i}"], np.float32) for i in range(4)]
    coords = np.asarray(inputs["coords"], np.float32)
    vis = np.asarray(inputs["vis"], np.float32)
    conf = np.asarray(inputs["conf"], np.float32)
    w1 = np.asarray(inputs["w1"], np.float32)
    b1 = np.asarray(inputs["b1"], np.float32)
    w2 = np.asarray(inputs["w2"], np.float32)
    b2 = np.asarray(inputs["b2"], np.float32)
    time_emb = np.asarray(inputs["time_emb"], np.float32)

    # ---- host staging ----
    sampT = _stage_sampled(fmaps, coords)          # (4, N, C, S, 49) f32

    # w1 viewed as (49 hw, 49 ij, 384) -> packed K chunks of 128
    w1v = w1.reshape(GG, GG, 384)
    w1p_full = np.zeros((C, NP25, 384), np.float32)
    w1p_full[0:49] = w1v[0:25].transpose(1, 0, 2)
    w1p_full[64:113, 0:24] = w1v[25:49].transpose(1, 0, 2)
    w1p_full = w1p_full.astype(BF16)
    w2s_full = np.ascontiguousarray(
        w2.reshape(3, 128, 256).transpose(1, 0, 2)).astype(BF16)
    b1s_full = np.ascontiguousarray(b1.reshape(3, 128).T).astype(np.float32)
    te_slice = time_emb[0, :, 2:1026] + np.tile(b2, 4)[None, :]   # (S, 1024)
    # te3[p, lvl, m2, s]
    te3_full = np.ascontiguousarray(
        te_slice.T.reshape(4, 2, 128, S).transpose(2, 0, 1, 3)).astype(np.float32)

    in_maps = []
    for k in range(NCORES):
        ns = slice(k * NPC, (k + 1) * NPC)
        # sampt: (4 lvl, 8 tile, C, 16 n, 400) cols hw*8+s (zero pad 392:400)
        sa = sampT[:, ns]                              # (4, NPC, C, S, GG)
        sa = sa.transpose(0, 2, 1, 4, 3)               # (lvl, c, n, hw, s)
        sa = sa.reshape(4, C, 8, NT, GG * S)
        sa = np.concatenate(
            [sa, np.zeros((4, C, 8, NT, 8), np.float32)], axis=-1)
        sa = np.ascontiguousarray(sa.transpose(0, 2, 1, 3, 4)).astype(BF16)
        # trackt: (4, C, NPC, 49); track lvl input (1, 49, N, C)
        tr = np.stack([
            np.ascontiguousarray(t[0][:, ns].transpose(2, 1, 0))
            for t in tracks
        ]).astype(BF16)
        in_maps.append({
            "sampt": sa,
            "trackt": tr,
            "w1p": w1p_full,
            "w2s": w2s_full,
            "b1s": b1s_full,
            "te3": te3_full,
        })

    # ---- device run ----
    from concourse import bass_utils
    global _LAST_INMAPS
    _LAST_INMAPS = in_maps
    if "nc" not in _NC_CACHE:
        _NC_CACHE["nc"] = _build_device_program()
    res = bass_utils.run_bass_kernel_spmd(
        _NC_CACHE["nc"], in_maps, list(range(NCORES)))
    results = res.results

    # ---- host tail: rel posenc + assembly ----
    rel_f = np.concatenate(
        [coords[:, :-1] - coords[:, 1:], np.zeros((1, 1, N, 2), np.float32)], axis=1)
    rel_b = np.concatenate(
        [np.zeros((1, 1, N, 2), np.float32), coords[:, 1:] - coords[:, :-1]], axis=1)
    scale = np.asarray([RES[1], RES[0]], np.float32) / STRIDE
    rel_emb = _posenc(np.concatenate(
        [rel_f / scale, rel_b / scale], axis=-1))     # (1, S, N, 84)

    out = np.empty((1, N, S, 1110), np.float32)
    te = time_emb[0]                                  # (S, 1110)
    out[0, :, :, 0] = vis[0, :, :, 0].T + te[None, :, 0]
    out[0, :, :, 1] = conf[0, :, :, 0].T + te[None, :, 1]
    out[0, :, :, 1026:] = rel_emb[0].transpose(1, 0, 2) + te[None, :, 1026:]
    for k in range(NCORES):
        # OUT[feat, tok]: feat = (lvl, m2, p), tok = (h, uu, pl, t, s);
        # track n = 32*(2h+uu) + 2pl + t
        o = np.asarray(results[k]["OUT"], np.float32)
        emb = o.reshape(1024, 2, 2, 16, 2, 8)          # (feat, h, uu, pl, t, s)
        emb = emb.transpose(1, 2, 3, 4, 5, 0).reshape(NPC, S, 1024)
        out[0, k * NPC:(k + 1) * NPC, :, 2:1026] = emb
    return out
